# revision 34
# baseline (speedup 1.0000x reference)
"""Trainium2 Bass kernel for nn_Attention_75402445849133.

Dense per-batch attention:
  q = Wq @ x[b] + bq ; k = Wk @ x[b] + bk ; v = x[b] (unprojected)
  per head h (16 heads, d=64, S=128):
    scores = (q_h^T k_h) / 8 ; attn = softmax(scores) ; out_h = attn @ v_h^T
  score[b, f] = sum_s out[f, s] * Wo[s] + bo

Sharded data-parallel over batch B=256 across 8 NeuronCores (32 b/core).
All matmul operands fp16 (fp32 PSUM accumulation).

Key tricks:
  - scores computed TRANSPOSED (t on partitions) so softmax denominator and
    the AV matmul both contract over t on partitions with no attn transpose.
  - x[b]^T passed from the host with a constant ones column appended after
    each head's 64 columns, so the AV matmul (stationary = exp(scores)) also
    emits the softmax denominator column from the same stationary load.
  - no max-subtraction in softmax (scores are O(1) by construction).
  - softmax division via DVE reciprocal + broadcast multiply (per-partition).
  - final f-projection = 2 column-group-concurrent matmuls with the Wo
    vector as stationary, into a persistent (memset-once) PSUM bank, with
    one partition-strided output DMA per batch.
  - bo added on host.
  - PSUM bank discipline: all matmuls targeting one bank share a PE row
    group (mixed-row-group writes to one bank are concurrent and fatal);
    the even/odd head score matmuls use separate banks + row groups and run
    concurrently (packed pairs).
  - scores for head-pair mt are emitted one mt behind the projection loop
    (only projection mt is needed), so the 16 exp() ops per group spread
    evenly across the Scalar engine instead of forming an exp-bound tail
    phase; attention output (uout) work drains one half-batch (2 parity
    quads = the 2 ps_uo slots) per mt-phase, one group deferred.
  - startup: bias/first-use weight/x chunks issued first across the
    gpsimd/scalar/sync DMA queues; ~5us of throwaway matmuls pre-warm the
    PE HAM clock gate (2.4GHz) while the first DMAs land; dummy matmuls in
    the epilogue keep it warm through the final sparse drains.
"""

import sys
import types

import numpy as np

from concourse import bass, bacc, bass_isa, tile, mybir
from concourse.bass_utils import run_bass_kernel_spmd


def _ensure_axon_hooks():
    """Provide antenv.axon_hooks if the image lacks it (needed for trace=True)."""
    try:
        import antenv.axon_hooks  # noqa: F401

        return
    except ImportError:
        pass
    import antenv

    mod = types.ModuleType("antenv.axon_hooks")
    mod._hook = None
    mod.set_axon_ntff_profile_hook = lambda h: setattr(mod, "_hook", h)
    mod.get_axon_ntff_profile_hook = lambda: mod._hook
    sys.modules["antenv.axon_hooks"] = mod
    antenv.axon_hooks = mod
    try:
        from trn_agent_boot.trn_boot import _ntff_profile_via_ctypes

        hook = _ntff_profile_via_ctypes("/opt/axon/libaxon_pjrt.so")
        if hook is not None:
            mod._hook = hook
    except Exception:
        pass


_ensure_axon_hooks()

F16 = mybir.dt.float16
F32 = mybir.dt.float32

N_CORES = 8
B = 256
F_IN = 1024
HID = 1024
H = 16
S = 128
D = 64  # head dim (both q/k and v)
KT = 8  # k tiles (F_IN / 128)
MT = 8  # m tiles (HID / 128)
TEMP = 8.0

TRACE = False  # test.py sets this for profiling runs


def build_bass(n_groups=8, G=4):
    """Build the per-core Bass graph. NB = n_groups * G local batches."""
    NB = n_groups * G
    NQK = G * S  # moving free dim of the QK matmuls

    nc = bacc.Bacc(None, target_bir_lowering=False)

    # host-prepared inputs (per core)
    xr = nc.dram_tensor("xr", [n_groups, 128, KT, G, S], F16, kind="ExternalInput")
    # x[b]^T per batch with a ones column after each head's 64 cols (baked on host)
    xtr = nc.dram_tensor("xtr", [n_groups, 128, G, H, D + 1], F16, kind="ExternalInput")
    wqt = nc.dram_tensor("wqt", [MT, 128, KT, 128], F16, kind="ExternalInput")
    wkt = nc.dram_tensor("wkt", [MT, 128, KT, 128], F16, kind="ExternalInput")
    bqr = nc.dram_tensor("bqr", [128, MT], F32, kind="ExternalInput")
    bkr = nc.dram_tensor("bkr", [128, MT], F32, kind="ExternalInput")
    wo16 = nc.dram_tensor("wo16", [128, 1], F16, kind="ExternalInput")
    out = nc.dram_tensor("out", [NB, 2, 512], F32, kind="ExternalOutput")

    with tile.TileContext(nc) as tc:
        with (
            tc.tile_pool(name="consts", bufs=1) as cpool,
            tc.tile_pool(name="xp", bufs=2) as xpool,
            tc.tile_pool(name="xtp", bufs=3) as xtpool,
            tc.tile_pool(name="qkp", bufs=2) as qkpool,
            tc.tile_pool(name="ep", bufs=36) as epool,
            tc.tile_pool(name="wfp", bufs=4) as wfpool,
            tc.tile_pool(name="uop", bufs=5) as uopool,
            tc.tile_pool(name="orow", bufs=2) as orowpool,
            # one shared 5-slot pool for QK accumulators and score tiles
            # (same 2KB/partition size, busy in different phases)
            tc.tile_pool(name="ps_qs", bufs=3, space="PSUM") as ps_qs,
            tc.tile_pool(name="ps_sc", bufs=2, space="PSUM") as ps_sc,
            tc.tile_pool(name="ps_uo", bufs=2, space="PSUM") as ps_uo,
            tc.tile_pool(name="ps_fp", bufs=1, space="PSUM") as ps_fp,
        ):
            # ---- persistent tiles ----
            # per-mt weight tiles so the first matmul only waits on one DMA
            wq_ts = [
                cpool.tile([128, KT, 128], F16, name=f"wq{mt}", tag=f"wq{mt}")
                for mt in range(MT)
            ]
            wk_ts = [
                cpool.tile([128, KT, 128], F16, name=f"wk{mt}", tag=f"wk{mt}")
                for mt in range(MT)
            ]
            bq_t = cpool.tile([128, MT], F32, tag="bq")
            bk_t = cpool.tile([128, MT], F32, tag="bk")
            wo_t = cpool.tile([128, 1], F16, tag="wo")
            zero_t = cpool.tile([128, 1], F32, tag="zero")
            ones_t = cpool.tile([128, 1], F16, tag="ones")
            warm_t = cpool.tile([128, 512], F16, tag="warm")

            nc.vector.memset(zero_t[:], 0.0)
            nc.vector.memset(ones_t[:], 1.0)
            nc.vector.memset(warm_t[:], 0.0)

            # persistent finals PSUM bank, memset once so the full-width DVE
            # copy in finalize() only ever reads initialized, owned bytes
            ps_f = ps_fp.tile([33, 512], F32, tag="psf")
            nc.vector.memset(ps_f[:], 0.0)

            # Head DMA order: bias/wo tiles first (tiny, the first DVE
            # bias-add must not stall), then wq0/x/wk0 chunks in first-use
            # order, split across the gpsimd/scalar/sync queues (descriptor
            # generation is ~620ns per dma_start per queue).
            x16_first = xpool.tile([128, KT, G, S], F16, tag="x16")
            nc.gpsimd.dma_start(bq_t[:], bqr[:])
            nc.gpsimd.dma_start(bk_t[:], bkr[:])
            nc.gpsimd.dma_start(wo_t[:], wo16[:])
            nc.gpsimd.dma_start(
                wq_ts[0][:, 0:4, :],
                wqt[0][:, 0:4, :].rearrange("p kt m -> p (kt m)"),
            )
            nc.scalar.dma_start(
                x16_first[:, 0:2, :, :],
                xr[0][:, 0:2].rearrange("p kt g s -> p (kt g s)"),
            )
            nc.gpsimd.dma_start(
                x16_first[:, 2:4, :, :],
                xr[0][:, 2:4].rearrange("p kt g s -> p (kt g s)"),
            )
            nc.sync.dma_start(
                wq_ts[0][:, 4:8, :],
                wqt[0][:, 4:8, :].rearrange("p kt m -> p (kt m)"),
            )
            nc.sync.dma_start(
                x16_first[:, 4:6, :, :],
                xr[0][:, 4:6].rearrange("p kt g s -> p (kt g s)"),
            )
            nc.sync.dma_start(
                x16_first[:, 6:8, :, :],
                xr[0][:, 6:8].rearrange("p kt g s -> p (kt g s)"),
            )
            nc.sync.dma_start(wk_ts[0][:], wkt[0].rearrange("p kt m -> p (kt m)"))
            for mt in range(1, MT):
                nc.sync.dma_start(
                    wq_ts[mt][:], wqt[mt].rearrange("p kt m -> p (kt m)")
                )
                nc.sync.dma_start(
                    wk_ts[mt][:], wkt[mt].rearrange("p kt m -> p (kt m)")
                )

            # PE pre-warm: throwaway matmuls during the initial DMA wait so
            # the HAM clock gate reaches 8/8 and stays there until real
            # matmuls have data (~5us of filler).
            ps_w = ps_qs.tile([128, NQK], F32, tag="qs")
            for i in range(24):
                nc.tensor.matmul(
                    ps_w[:], warm_t[:, 0:128], warm_t[:],
                    start=(i == 0), stop=(i == 23),
                )

            attn_pending = []

            pending = []

            uo_by_batch = {}

            def do_uout(entry):
                # one oc-half of one batch: 2 parity quads = exactly the two
                # ps_uo slots, so a drain block never waits on the previous
                # block's DVE normalize (blocks are a full mt-phase apart)
                ub, g, xT, Ed, oc = entry
                if oc == 0:
                    uo_sc = uopool.tile([128, H * D], F16, tag="uosc")
                    uo_by_batch[ub] = uo_sc
                else:
                    uo_sc = uo_by_batch.pop(ub)
                uo_view = uo_sc[:].rearrange(
                    "p (pair par d) -> p pair par d", par=2, d=D
                )
                for par in range(2):
                    ps_u = ps_uo.tile([128, 4, D + 1], F32, tag="uo")
                    for j in range(4):
                        h = 2 * (oc * 4 + j) + par
                        # uoutT[s, 0:64] = sum_t E[t,s] * xT[t, d]
                        # uoutT[s, 64]   = sum_t E[t,s]  (ones col)
                        nc.tensor.matmul(
                            ps_u[:, j, :],
                            Ed[(oc * 4 + j, par)][:, g, :],
                            xT[:, h, :],
                        )
                    # rc[s, j] = 1 / colsum_j[s]
                    rc = wfpool.tile([128, 4], F32, tag="rc")
                    nc.vector.reciprocal(rc[:], ps_u[:, :, D])
                    # uo_sc[s, (h,d)] = uoutT[s, (h,d)] * rc[s, h]
                    nc.vector.tensor_mul(
                        uo_view[:, oc * 4 : (oc + 1) * 4, par, :],
                        ps_u[:, :, 0:D],
                        rc[:].unsqueeze(2).broadcast_to((128, 4, D)),
                    )
                if oc == 1:
                    # PE finals, deferred one batch so the PE never waits on
                    # the DVE normalization chain
                    pending.append((ub, uo_sc))
                    if len(pending) > 1:
                        finalize(pending.pop(0))

            def finalize(item):
                # final projection: score[f] = sum_s uo_sc[s, f] (Wo folded
                # into uo_sc already). The two N=512 matmuls land on
                # partitions 0/32 of one PSUM bank and run concurrently;
                # one gathered DMA (partition-strided) writes both halves.
                fb, uo = item
                nc.tensor.matmul(ps_f[0:1, :], wo_t[:], uo[:, 0:512])
                nc.tensor.matmul(ps_f[32:33, :], wo_t[:], uo[:, 512:1024])
                orow = orowpool.tile([33, 512], F32, tag="orow")
                nc.vector.tensor_copy(orow[:], ps_f[:])
                nc.sync.dma_start(out[fb], orow[0:33:32, :])

            for grp in range(n_groups):
                # ---- load x group + transposed x for the group's batches ----
                if grp == 0:
                    x16 = x16_first
                else:
                    x16 = xpool.tile([128, KT, G, S], F16, tag="x16")
                    nc.sync.dma_start(
                        x16[:], xr[grp].rearrange("p kt g s -> p (kt g s)")
                    )
                xT4 = xtpool.tile([128, G, H, D + 1], F16, tag="xT")
                nc.sync.dma_start(
                    xT4[:], xtr[grp].rearrange("p g h d -> p (g h d)")
                )

                # ---- QK projections interleaved with per-mt scores ----
                # Scores for head-pair mt (even head 2mt in PE rows 0-63,
                # odd head 2mt+1 in rows 64-127) need only projection mt, so
                # they slot into the projection stream one mt behind; the two
                # exps per mt then spread evenly across the whole group on
                # ACT instead of clumping into an exp-bound tail phase.
                q_sb = qkpool.tile([128, MT, NQK], F16, tag="q")
                k_sb = qkpool.tile([128, MT, NQK], F16, tag="k")
                E_cur = {}

                def emit_scores(smt):
                    # scoresT[t, s] for all 4 batches of head-pair smt.
                    # Parity pair packs: different PE row groups, separate
                    # PSUM banks -> concurrent.
                    ps_e = ps_sc.tile([128, G, S], F32, tag="sc", name="ps_e")
                    ps_o = ps_sc.tile([128, G, S], F32, tag="sc", name="ps_o")
                    for g in range(G):
                        for par, ps_s in ((0, ps_e), (1, ps_o)):
                            po = par * D
                            nc.tensor.matmul(
                                ps_s[:, g, :],
                                k_sb[po : po + D, smt, g * S : (g + 1) * S],
                                q_sb[po : po + D, smt, g * S : (g + 1) * S],
                            )
                    for par, ps_s in ((0, ps_e), (1, ps_o)):
                        E = epool.tile([128, G, S], F16, tag="E", name="E")
                        nc.scalar.activation(
                            E[:],
                            ps_s[:],
                            mybir.ActivationFunctionType.Exp,
                            bias=zero_t[:, 0:1],
                            scale=1.0 / TEMP,
                        )
                        E_cur[(smt, par)] = E

                for mt in range(MT):
                    for pi, (w_ts, b_t, dst) in enumerate((
                        (wq_ts, bq_t, q_sb),
                        (wk_ts, bk_t, k_sb),
                    )):
                        ps = ps_qs.tile([128, NQK], F32, tag="qs")
                        for kt in range(KT):
                            nc.tensor.matmul(
                                ps[:],
                                w_ts[mt][:, kt, :],
                                x16[:, kt, :, :].rearrange("p g s -> p (g s)"),
                                start=(kt == 0),
                                stop=(kt == KT - 1),
                            )
                        # bias add + fp16 cast (DVE)
                        nc.vector.tensor_scalar_add(
                            dst[:, mt, :], ps[:], b_t[:, mt : mt + 1]
                        )
                        # last group: double the drain rate (extra point
                        # between the q and k projections) so its own oc=0
                        # halves drain in-loop instead of in the epilogue
                        if (
                            pi == 0
                            and grp == n_groups - 1
                            and mt >= 1
                            and attn_pending
                        ):
                            do_uout(attn_pending.pop(0))
                    if mt >= 1:
                        # one mt behind so the DVE bias-add latency is hidden
                        # behind the current mt's projection matmuls
                        emit_scores(mt - 1)
                        # in the last group the oc=0 halves can drain inside
                        # the group itself (their E tiles exist after s3)
                        if grp == n_groups - 1 and mt == 5:
                            for g in range(G):
                                attn_pending.append(
                                    (grp * G + g, g, xT4[:, g], E_cur, 0)
                                )
                        # one oc-half of one previous batch per mt-phase
                        if attn_pending:
                            do_uout(attn_pending.pop(0))
                emit_scores(MT - 1)
                for g in range(G):
                    for oc in range(2):
                        if grp == n_groups - 1 and oc == 0:
                            continue  # already queued at mt==5
                        attn_pending.append(
                            (grp * G + g, g, xT4[:, g], E_cur, oc)
                        )
                if len(attn_pending) > 8:
                    do_uout(attn_pending.pop(0))

            while attn_pending:
                do_uout(attn_pending.pop(0))
                if attn_pending:
                    # dummy matmuls keep the HAM clock gate at 8/8 through
                    # the sparse epilogue (and fill the ps_uo WAR wait)
                    for i in range(4):
                        nc.tensor.matmul(
                            ps_w[:], warm_t[:, 0:128], warm_t[:],
                            start=(i == 0), stop=(i == 3),
                        )
            while pending:
                finalize(pending.pop(0))

    nc.compile()
    return nc


def prep_inputs(x, Wq, bq, Wk, bk, Wo, n_groups=8, G=4, n_cores=N_CORES):
    """Host-side shard + layout prep. Returns in_maps for run_bass_kernel_spmd."""
    x = np.asarray(x, dtype=np.float32)
    nb = n_groups * G
    x16 = x.astype(np.float16)
    # (c, grp, g, kt, p, s) -> (c, grp, p, kt, g, s)
    xr = (
        x16.reshape(n_cores, n_groups, G, KT, 128, S)
        .transpose(0, 1, 4, 3, 2, 5)
        .copy()
    )
    # x^T per batch with ones col per head: (c, grp, t, g, h, 65)
    xtr = np.ones((n_cores, n_groups, S, G, H, D + 1), dtype=np.float16)
    xtr[..., 0:D] = x16.reshape(n_cores, n_groups, G, H, D, S).transpose(
        0, 1, 5, 2, 3, 4
    )
    # W.T is (k, m); lay out as (mt, p, kt, 128) so each mt tile is one DMA
    wqt = np.ascontiguousarray(
        np.asarray(Wq, dtype=np.float32).T.reshape(KT, 128, MT, 128).transpose(2, 1, 0, 3)
    ).astype(np.float16)
    wkt = np.ascontiguousarray(
        np.asarray(Wk, dtype=np.float32).T.reshape(KT, 128, MT, 128).transpose(2, 1, 0, 3)
    ).astype(np.float16)
    bqr = np.ascontiguousarray(np.asarray(bq, dtype=np.float32).reshape(MT, 128).T)
    bkr = np.ascontiguousarray(np.asarray(bk, dtype=np.float32).reshape(MT, 128).T)
    wo_a = np.asarray(Wo, dtype=np.float32).reshape(128, 1).astype(np.float16)
    in_maps = []
    for c in range(n_cores):
        in_maps.append(
            {
                "xr": xr[c],
                "xtr": xtr[c],
                "wqt": wqt,
                "wkt": wkt,
                "bqr": bqr,
                "bkr": bkr,
                "wo16": wo_a,
            }
        )
    return in_maps


_NC_CACHE = {}


def kernel(x, Wq, bq, Wk, bk, Wo, bo):
    key = "full"
    if key not in _NC_CACHE:
        _NC_CACHE[key] = build_bass()
    nc = _NC_CACHE[key]

    in_maps = prep_inputs(x, Wq, bq, Wk, bk, Wo)
    res = run_bass_kernel_spmd(nc, in_maps, core_ids=list(range(N_CORES)), trace=TRACE)
    kernel.last_result = res
    out = np.concatenate(
        [res.results[c]["out"].reshape(-1, F_IN) for c in range(N_CORES)], axis=0
    )
    out = out + np.float32(np.asarray(bo).reshape(-1)[0])
    return out.astype(np.float32)

